# revision 1
# baseline (speedup 1.0000x reference)
"""Trainium2 Bass kernel for nn_DecoderLayer_90967407329666.

Decoder layer: LN1 -> QKV (+type emb) -> multi-axis RoPE -> causal SDPA
-> residual -> LN2 -> SwiGLU FFN -> residual.  B=2, T=2048, D=768, H=8,
DFF=2048, NTYPE=16, NAX=2 rotary axes of 32 dims each.

Sharding (8 cores):
  Phase 1 (token-parallel): each core owns 512 tokens (core c: batch c//4,
    tokens 512*(c%4)...) and computes LN1 + q,k (feature-major) + v
    (token-major) + type-emb + RoPE for those tokens, all 8 heads.
  AllToAll #1: block j carries head j's q,k,v -> core c ends up with head c
    for ALL 4096 tokens (static offsets, same program on every core).
  Phase 2 (head-parallel): core c runs full causal attention for head c,
    both batches (feature-major scores, softmax sums via an appended
    ones-column on v, exp without max-subtraction -- scores are O(5)).
  AllToAll #2: o goes back token-parallel.
  Phase 3 (token-parallel): residual + LN2 + SwiGLU FFN + residual for the
    core's 512 tokens.

All matmuls run as float32r (full PE rate at N>=256, ~12-bit mantissa);
the residual stream stays float32.
"""

import sys

sys.path.insert(0, "/opt/trn_rl_repo")

import numpy as np

import contextlib

import concourse.bacc as bacc
import concourse.bass as bass
import concourse.tile as tile
from concourse import mybir
from concourse.bass_utils import run_bass_kernel_spmd

# ---- problem constants (hardcoded per contest rules) ----
B, T = 2, 2048
D, H, NAX, DFF, NTYPE = 768, 8, 2048 // 1, 2048, 16  # NAX fixed below
NAX = 2
HD = D // H            # 96
DR = HD // (NAX + 1)   # 32
EPS = 1e-5
THETA = 10000.0
N_CORES = 8
TPC = 512              # tokens per core
NSUP = 4               # supertiles per batch (2048/512)
KD = D // 128          # 6 contraction chunks over D
SCALE = 1.0 / np.sqrt(np.float32(HD))

F32 = mybir.dt.float32
F32R = mybir.dt.float32r
I32 = mybir.dt.int32

# Cody-Waite split of 2*pi (C1 has 12 mantissa bits -> n*C1 exact for n<2^11)
C1 = float(np.float32(np.floor(2 * np.pi * 2**9) / 2**9))
C2 = float(np.float32(2 * np.pi - C1))
C3 = float(np.float32(2 * np.pi - C1 - float(np.float32(2 * np.pi - C1))))
HALF_PI = float(np.pi / 2)

# qk output-feature permutation: 12 slices of 128 rows
#   slices 0..7  : [q_h dims 0:64 | k_h dims 0:64]   (rope rows)
#   slice  8, 9  : q tails (dims 64:96) of heads 0..3 / 4..7
#   slice 10,11  : k tails of heads 0..3 / 4..7
def _qk_colperm():
    cols = []
    for h in range(H):
        cols += list(range(96 * h, 96 * h + 64))          # q_h 0:64
        cols += list(range(768 + 96 * h, 768 + 96 * h + 64))  # k_h 0:64
    for h in range(H):
        cols += list(range(96 * h + 64, 96 * h + 96))     # q tails
    for h in range(H):
        cols += list(range(768 + 96 * h + 64, 768 + 96 * h + 96))  # k tails
    return np.array(cols)

QK_PERM = _qk_colperm()

_prog_cache = {}


def build_program(payload_bf16=False):
    key = (payload_bf16,)
    if key in _prog_cache:
        return _prog_cache[key]
    nc = bacc.Bacc("TRN2", target_bir_lowering=False, debug=False,
                   num_devices=N_CORES)
    alu = mybir.AluOpType
    act = mybir.ActivationFunctionType

    # ---------------- DRAM I/O ----------------
    xT_d = nc.dram_tensor("xT", [D, TPC], F32R, kind="ExternalInput")
    wqk_d = nc.dram_tensor("Wqk", [D, 1536], F32R, kind="ExternalInput")
    wv_d = nc.dram_tensor("Wv", [D, D], F32R, kind="ExternalInput")
    teq_d = nc.dram_tensor("te_q", [NTYPE, 1536], F32R, kind="ExternalInput")
    tek_d = nc.dram_tensor("te_k", [NTYPE, 1536], F32R, kind="ExternalInput")
    qtype_d = nc.dram_tensor("qtype", [1, TPC], F32R, kind="ExternalInput")
    ktype_d = nc.dram_tensor("ktype", [1, TPC], F32R, kind="ExternalInput")
    pos4_d = nc.dram_tensor("pos4", [4, TPC], F32R, kind="ExternalInput")
    invf_d = nc.dram_tensor("invf", [128, 1], F32, kind="ExternalInput")
    g1_d = nc.dram_tensor("g1c", [128, KD], F32, kind="ExternalInput")
    b1_d = nc.dram_tensor("b1c", [128, KD], F32, kind="ExternalInput")
    g2_d = nc.dram_tensor("g2c", [128, KD], F32, kind="ExternalInput")
    b2_d = nc.dram_tensor("b2c", [128, KD], F32, kind="ExternalInput")
    w1_d = nc.dram_tensor("W1", [D, 2 * DFF], F32R, kind="ExternalInput")
    w2_d = nc.dram_tensor("W2", [DFF, D], F32R, kind="ExternalInput")
    b1a_d = nc.dram_tensor("b1a", [128, 16], F32, kind="ExternalInput")
    b1g_d = nc.dram_tensor("b1g", [128, 16], F32, kind="ExternalInput")
    bf2_d = nc.dram_tensor("bf2", [128, KD], F32, kind="ExternalInput")
    masks_d = nc.dram_tensor("masks", [128, 4 * 512], F32, kind="ExternalInput")
    r128_d = nc.dram_tensor("R128", [128, 128], F32R, kind="ExternalInput")
    b4_d = nc.dram_tensor("B4", [4, 128], F32R, kind="ExternalInput")
    ones_d = nc.dram_tensor("ones128", [1, 128], F32R, kind="ExternalInput")
    onescol_d = nc.dram_tensor("onescol", [128, 1], F32R, kind="ExternalInput")
    iota_d = nc.dram_tensor("iota16", [16, 1], F32, kind="ExternalInput")
    eps_d = nc.dram_tensor("epsc", [1, 1], F32, kind="ExternalInput")
    outT_d = nc.dram_tensor("outT", [D, TPC], F32, kind="ExternalOutput")

    with tile.TileContext(nc) as tc:
        with tc.tile_pool(name="glob", bufs=1) as glob, \
             tc.tile_pool(name="dram", bufs=1, space="DRAM") as dram:
            # exchange slabs
            slab_in = dram.tile([1536, TPC], F32R, tag="slab_in")
            slab_out = dram.tile([1536, TPC], F32R, tag="slab_out")
            slabv_in = dram.tile([768, TPC], F32R, tag="slabv_in")
            slabv_out = dram.tile([768, TPC], F32R, tag="slabv_out")
            slab2_in = dram.tile([D, TPC], F32, tag="slab2_in")
            slab2_out = dram.tile([D, TPC], F32, tag="slab2_out")

            # ---- persistent constants / activations ----
            ones_sb = glob.tile([1, 128], F32R, tag="ones")
            nc.sync.dma_start(out=ones_sb[:], in_=ones_d[:])
            onescol_sb = glob.tile([128, 1], F32R, tag="onescol")
            nc.sync.dma_start(out=onescol_sb[:], in_=onescol_d[:])
            iota_sb = glob.tile([16, 1], F32, tag="iota")
            nc.sync.dma_start(out=iota_sb[:], in_=iota_d[:])
            eps_sb = glob.tile([1, 1], F32, tag="eps")
            nc.sync.dma_start(out=eps_sb[:], in_=eps_d[:])
            g1_sb = glob.tile([128, KD], F32, tag="g1")
            nc.sync.dma_start(out=g1_sb[:], in_=g1_d[:])
            b1_sb = glob.tile([128, KD], F32, tag="b1")
            nc.sync.dma_start(out=b1_sb[:], in_=b1_d[:])
            g2_sb = glob.tile([128, KD], F32, tag="g2")
            nc.sync.dma_start(out=g2_sb[:], in_=g2_d[:])
            b2_sb = glob.tile([128, KD], F32, tag="b2")
            nc.sync.dma_start(out=b2_sb[:], in_=b2_d[:])
            xT = []
            for k in range(KD):
                t = glob.tile([128, TPC], F32R, tag=f"xT{k}")
                nc.sync.dma_start(out=t[:], in_=xT_d[128 * k:128 * (k + 1), :])
                xT.append(t)

            def layernorm_stats(pool, src_tiles, tag):
                """src (fp32r, 6 x (128,TPC)) -> (alpha_row, beta_row) SBUF."""
                ctx = contextlib.ExitStack()
                ps_pool = ctx.enter_context(
                    tc.tile_pool(name=f"{tag}ps", bufs=1, space="PSUM"))
                sums = ps_pool.tile([1, TPC], F32, tag=f"{tag}sums")
                sumsq = ps_pool.tile([1, TPC], F32, tag=f"{tag}sumsq")
                for k in range(KD):
                    sq = pool.tile([128, TPC], F32R, tag=f"{tag}sq")
                    nc.vector.tensor_tensor(
                        out=sq[:], in0=src_tiles[k][:].bitcast(F32),
                        in1=src_tiles[k][:].bitcast(F32), op=alu.mult)
                    nc.tensor.matmul(sums[:], onescol_sb[:], src_tiles[k][:],
                                     start=(k == 0), stop=(k == KD - 1))
                    nc.tensor.matmul(sumsq[:], onescol_sb[:], sq[:],
                                     start=(k == 0), stop=(k == KD - 1))
                mean = pool.tile([1, TPC], F32, tag=f"{tag}mean")
                nc.vector.tensor_scalar(out=mean[:], in0=sums[:],
                                        scalar1=1.0 / D, scalar2=None,
                                        op0=alu.mult)
                m2 = pool.tile([1, TPC], F32, tag=f"{tag}m2")
                nc.vector.tensor_tensor(out=m2[:], in0=mean[:], in1=mean[:],
                                        op=alu.mult)
                var = pool.tile([1, TPC], F32, tag=f"{tag}var")
                nc.vector.scalar_tensor_tensor(
                    out=var[:], in0=sumsq[:], scalar=1.0 / D, in1=m2[:],
                    op0=alu.mult, op1=alu.subtract)
                std = pool.tile([1, TPC], F32, tag=f"{tag}std")
                nc.scalar.activation(out=std[:], in_=var[:], func=act.Sqrt,
                                     bias=eps_sb[:])
                alpha = pool.tile([1, TPC], F32R, tag=f"{tag}alpha")
                with nc.allow_low_precision(reason="fp32r bcast rhs"):
                    nc.vector.reciprocal(out=alpha[:], in_=std[:])
                beta = pool.tile([1, TPC], F32R, tag=f"{tag}beta")
                nc.vector.scalar_tensor_tensor(
                    out=beta[:], in0=mean[:], scalar=-1.0,
                    in1=alpha[:].bitcast(F32), op0=alu.mult, op1=alu.mult)
                ctx.close()
                return alpha, beta

            def ln_apply(pool, scratch, src_f32_aps, alpha, beta, g_sb, b_sb,
                         tag, out_dt=F32R):
                """out[k] = (src*alpha)*g + (beta*g + b), 6 fp32r tiles."""
                ctx = contextlib.ExitStack()
                ps_pool = ctx.enter_context(
                    tc.tile_pool(name=f"{tag}ps", bufs=1, space="PSUM"))
                ab = ps_pool.tile([128, TPC], F32, tag=f"{tag}ab")
                nc.tensor.matmul(ab[:], ones_sb[:], alpha[:], start=True,
                                 stop=True)
                bb = ps_pool.tile([128, TPC], F32, tag=f"{tag}bb")
                nc.tensor.matmul(bb[:], ones_sb[:], beta[:], start=True,
                                 stop=True)
                ab_sb = pool.tile([128, TPC], F32, tag=f"{tag}absb")
                nc.vector.tensor_copy(out=ab_sb[:], in_=ab[:])
                bb_sb = pool.tile([128, TPC], F32, tag=f"{tag}bbsb")
                nc.vector.tensor_copy(out=bb_sb[:], in_=bb[:])
                ctx.close()
                outs = []

                for k in range(KD):
                    bbp = scratch.tile([128, TPC], F32, tag=f"{tag}bbp")
                    nc.vector.tensor_scalar(
                        out=bbp[:], in0=bb_sb[:], scalar1=g_sb[:, k:k + 1],
                        scalar2=b_sb[:, k:k + 1], op0=alu.mult, op1=alu.add)
                    t1 = scratch.tile([128, TPC], F32, tag=f"{tag}t1")
                    nc.vector.scalar_tensor_tensor(
                        out=t1[:], in0=src_f32_aps[k], scalar=g_sb[:, k:k + 1],
                        in1=ab_sb[:], op0=alu.mult, op1=alu.mult)
                    o = pool.tile([128, TPC], out_dt, tag=f"{tag}o{k}")
                    nc.vector.tensor_tensor(out=o[:], in0=t1[:], in1=bbp[:],
                                            op=alu.add)
                    outs.append(o)
                return outs

            # ================= PHASE 1 =================
            with contextlib.ExitStack() as p1:
                p1w = p1.enter_context(tc.tile_pool(name="p1w", bufs=1))
                p1t = p1.enter_context(tc.tile_pool(name="p1t", bufs=2))
                p1qk = p1.enter_context(
                    tc.tile_pool(name="p1qk", bufs=2, space="PSUM"))

                wqk = []
                for k in range(KD):
                    t = p1w.tile([128, 1536], F32R, tag=f"wqk{k}")
                    nc.sync.dma_start(out=t[:],
                                      in_=wqk_d[128 * k:128 * (k + 1), :])
                    wqk.append(t)
                wv = []
                for k in range(KD):
                    t = p1w.tile([128, D], F32R, tag=f"wv{k}")
                    nc.sync.dma_start(out=t[:],
                                      in_=wv_d[128 * k:128 * (k + 1), :])
                    wv.append(t)
                teq_sb = p1w.tile([NTYPE, 1536], F32R, tag="teq")
                nc.sync.dma_start(out=teq_sb[:], in_=teq_d[:])
                tek_sb = p1w.tile([NTYPE, 1536], F32R, tag="tek")
                nc.sync.dma_start(out=tek_sb[:], in_=tek_d[:])
                r128_sb = p1w.tile([128, 128], F32R, tag="r128")
                nc.sync.dma_start(out=r128_sb[:], in_=r128_d[:])
                b4_sb = p1w.tile([4, 128], F32R, tag="b4")
                nc.sync.dma_start(out=b4_sb[:], in_=b4_d[:])
                invf_sb = p1w.tile([128, 1], F32, tag="invf")
                nc.sync.dma_start(out=invf_sb[:], in_=invf_d[:])
                pos4_sb = p1w.tile([4, TPC], F32R, tag="pos4")
                nc.sync.dma_start(out=pos4_sb[:], in_=pos4_d[:])
                qt_sb = p1w.tile([1, TPC], F32R, tag="qt")
                nc.sync.dma_start(out=qt_sb[:], in_=qtype_d[:])
                kt_sb = p1w.tile([1, TPC], F32R, tag="kt")
                nc.sync.dma_start(out=kt_sb[:], in_=ktype_d[:])

                # LN1
                a1, be1 = layernorm_stats(p1t, xT, "l1")
                xn = ln_apply(p1w, p1t, [x[:].bitcast(F32) for x in xT],
                              a1, be1, g1_sb, b1_sb, "l1a")


                # one-hot type codes (16, TPC)
                p1misc = p1.enter_context(
                    tc.tile_pool(name="p1misc", bufs=1, space="PSUM"))

                def onehot(row_sb, tag):
                    bc = p1misc.tile([16, TPC], F32, tag="ohbc")
                    nc.tensor.matmul(bc[:], ones_sb[:, 0:16], row_sb[:],
                                     start=True, stop=True)
                    oh = p1w.tile([16, TPC], F32R, tag=f"{tag}oh")
                    nc.vector.tensor_scalar(out=oh[:], in0=bc[:],
                                            scalar1=iota_sb[:], scalar2=None,
                                            op0=alu.is_equal)
                    return oh
                oh_q = onehot(qt_sb, "q")
                oh_k = onehot(kt_sb, "k")

                # cos/sin tiles (128, TPC): rows 0:64 q-axes, 64:128 k-axes
                pm = p1misc.tile([128, TPC], F32, tag="pm")
                nc.tensor.matmul(pm[:], b4_sb[:], pos4_sb[:], start=True,
                                 stop=True)
                f_t = p1t.tile([128, TPC], F32, tag="f")
                nc.vector.tensor_scalar(out=f_t[:], in0=pm[:],
                                        scalar1=invf_sb[:], scalar2=None,
                                        op0=alu.mult)
                nt = p1t.tile([128, TPC], F32, tag="nt")
                nc.vector.tensor_scalar(out=nt[:], in0=f_t[:],
                                        scalar1=float(1.0 / (2 * np.pi)),
                                        scalar2=None, op0=alu.mult)
                n_i = p1t.tile([128, TPC], I32, tag="ni")
                nc.vector.tensor_copy(out=n_i[:], in_=nt[:])
                n_f = p1t.tile([128, TPC], F32, tag="nf")
                nc.vector.tensor_copy(out=n_f[:], in_=n_i[:])
                fr = p1t.tile([128, TPC], F32, tag="fr")
                nc.vector.scalar_tensor_tensor(out=fr[:], in0=n_f[:],
                                               scalar=-C1, in1=f_t[:],
                                               op0=alu.mult, op1=alu.add)
                nc.vector.scalar_tensor_tensor(out=fr[:], in0=n_f[:],
                                               scalar=-C2, in1=fr[:],
                                               op0=alu.mult, op1=alu.add)
                nc.vector.scalar_tensor_tensor(out=fr[:], in0=n_f[:],
                                               scalar=-C3, in1=fr[:],
                                               op0=alu.mult, op1=alu.add)
                s_t = p1w.tile([128, TPC], F32, tag="sin")
                nc.scalar.activation(out=s_t[:], in_=fr[:], func=act.Sin)
                af = p1t.tile([128, TPC], F32, tag="af")
                nc.scalar.activation(out=af[:], in_=fr[:], func=act.Abs)
                ca = p1t.tile([128, TPC], F32, tag="ca")
                nc.vector.tensor_scalar(out=ca[:], in0=af[:], scalar1=-1.0,
                                        scalar2=HALF_PI, op0=alu.mult,
                                        op1=alu.add)
                c_t = p1w.tile([128, TPC], F32, tag="cos")
                nc.scalar.activation(out=c_t[:], in_=ca[:], func=act.Sin)
                # fold score scale 1/sqrt(HD) into q: scale c,s rows 0:64
                nc.vector.tensor_scalar(out=c_t[0:64, :], in0=c_t[0:64, :],
                                        scalar1=float(SCALE), scalar2=None,
                                        op0=alu.mult)
                nc.vector.tensor_scalar(out=s_t[0:64, :], in0=s_t[0:64, :],
                                        scalar1=float(SCALE), scalar2=None,
                                        op0=alu.mult)

                # qk slices: matmul + type emb, then rope / tails -> slab
                for s in range(12):
                    qk_ps = p1qk.tile([128, TPC], F32, tag="qkps")
                    for k in range(KD):
                        nc.tensor.matmul(qk_ps[:],
                                         wqk[k][:, 128 * s:128 * (s + 1)],
                                         xn[k][:], start=(k == 0), stop=False)
                    nc.tensor.matmul(qk_ps[:],
                                     teq_sb[:, 128 * s:128 * (s + 1)],
                                     oh_q[:], start=False, stop=False)
                    nc.tensor.matmul(qk_ps[:],
                                     tek_sb[:, 128 * s:128 * (s + 1)],
                                     oh_k[:], start=False, stop=True)
                    if s < 8:
                        # rope: q_h 0:64 | k_h 0:64
                        rsb = p1t.tile([128, TPC], F32R, tag="rsb")
                        nc.vector.tensor_copy(out=rsb[:], in_=qk_ps[:])
                        rot = p1qk.tile([128, TPC], F32, tag="rot")
                        nc.tensor.matmul(rot[:], r128_sb[:], rsb[:],
                                         start=True, stop=True)
                        t1 = p1t.tile([128, TPC], F32, tag="rt1")
                        nc.vector.tensor_tensor(out=t1[:],
                                                in0=rsb[:].bitcast(F32),
                                                in1=c_t[:], op=alu.mult)
                        t2 = p1t.tile([128, TPC], F32, tag="rt2")
                        nc.vector.tensor_tensor(out=t2[:], in0=rot[:],
                                                in1=s_t[:], op=alu.mult)
                        qkr = p1t.tile([128, TPC], F32R, tag="qkr")
                        nc.vector.tensor_tensor(out=qkr[:], in0=t1[:],
                                                in1=t2[:], op=alu.add)
                        h = s
                        nc.scalar.dma_start(
                            out=slab_in[192 * h + 0:192 * h + 64, :],
                            in_=qkr[0:64, :])
                        nc.scalar.dma_start(
                            out=slab_in[192 * h + 96:192 * h + 160, :],
                            in_=qkr[64:128, :])
                    else:
                        # tails: s=8,9 q tails h0..3/h4..7 (scale by 1/sqrt(HD))
                        # s=10,11 k tails
                        tl = p1t.tile([128, TPC], F32R, tag="tail")
                        sc = float(SCALE) if s < 10 else 1.0
                        nc.vector.tensor_scalar(out=tl[:], in0=qk_ps[:],
                                                scalar1=sc, scalar2=None,
                                                op0=alu.mult)
                        base = 64 if s < 10 else 160  # q tail at +64, k at +160
                        for j in range(4):
                            h = 4 * (s % 2) + j
                            nc.scalar.dma_start(
                                out=slab_in[192 * h + base:192 * h + base + 32, :],
                                in_=tl[32 * j:32 * (j + 1), :])

                nc.gpsimd.collective_compute(
                    "AllToAll", mybir.AluOpType.bypass,
                    replica_groups=[list(range(N_CORES))],
                    ins=[slab_in[:].bitcast(F32).opt()],
                    outs=[slab_out[:].bitcast(F32).opt()])

                # v (token-major): 4 tok-slices x 2 halves of 384 cols
                for ts_ in range(4):
                    for hf in range(2):
                        v_ps = p1qk.tile([128, 384], F32, tag="vps")
                        for k in range(KD):
                            nc.tensor.matmul(
                                v_ps[:],
                                xn[k][:, 128 * ts_:128 * (ts_ + 1)],
                                wv[k][:, 384 * hf:384 * (hf + 1)],
                                start=(k == 0), stop=(k == KD - 1))
                        v_sb1 = p1t.tile([128, 384], F32R, tag="vsb1")
                        nc.vector.tensor_copy(out=v_sb1[:], in_=v_ps[:])
                        for j in range(4):
                            h = 4 * hf + j
                            # v region of block h: rows 192:288 = (TPC x 96)
                            # row-major; tok-slice ts_ -> offset 128*ts_*96
                            dst = bass.AP(
                                tensor=slabv_in[:].tensor,
                                offset=(96 * h) * TPC + 128 * ts_ * 96,
                                ap=[[96, 128], [1, 96]])
                            nc.scalar.dma_start(
                                out=dst,
                                in_=v_sb1[:, 96 * j:96 * (j + 1)])

                nc.gpsimd.collective_compute(
                    "AllToAll", mybir.AluOpType.bypass,
                    replica_groups=[list(range(N_CORES))],
                    ins=[slabv_in[:].bitcast(F32).opt()],
                    outs=[slabv_out[:].bitcast(F32).opt()])

            # ================= PHASE 2 =================
            with contextlib.ExitStack() as p2:
                p2w = p2.enter_context(tc.tile_pool(name="p2w", bufs=1))
                p2t = p2.enter_context(tc.tile_pool(name="p2t", bufs=3))
                p2ps = p2.enter_context(
                    tc.tile_pool(name="p2ps", bufs=4, space="PSUM"))
                p2o = p2.enter_context(
                    tc.tile_pool(name="p2o", bufs=2, space="PSUM"))
                p2rb = p2.enter_context(
                    tc.tile_pool(name="p2rb", bufs=1, space="PSUM"))

                masks_sb = p2w.tile([128, 4 * 512], F32, tag="masks")
                nc.sync.dma_start(out=masks_sb[:], in_=masks_d[:])
                den_all = p2w.tile([97, 8 * 512], F32, tag="denall")
                rec8 = p2w.tile([8, 512], F32R, tag="rec8")
                o_all = {}

                for bb_ in range(2):
                    qT = p2w.tile([96, 2048], F32R, tag=f"qT{bb_}")
                    kT = p2w.tile([96, 2048], F32R, tag=f"kT{bb_}")
                    v_sb = p2w.tile([128, 16, 97], F32R, tag=f"v{bb_}")
                    ones_bc = bass.AP(
                        tensor=onescol_sb[:].tensor,
                        offset=onescol_sb[:].offset,
                        ap=[[1, 128], [0, 16], [0, 1]])
                    nc.sync.dma_start(out=v_sb[:, :, 96:97], in_=ones_bc)
                    for u in range(4):
                        blk = 192 * (4 * bb_ + u)
                        nc.sync.dma_start(
                            out=qT[:, 512 * u:512 * (u + 1)],
                            in_=slab_out[blk + 0:blk + 96, :])
                        nc.sync.dma_start(
                            out=kT[:, 512 * u:512 * (u + 1)],
                            in_=slab_out[blk + 96:blk + 192, :])
                        for ts_ in range(4):
                            src = bass.AP(
                                tensor=slabv_out[:].tensor,
                                offset=96 * (4 * bb_ + u) * TPC + 128 * ts_ * 96,
                                ap=[[96, 128], [1, 96]])
                            nc.sync.dma_start(
                                out=v_sb[:, 4 * u + ts_, 0:96], in_=src)

                    for Q in reversed(range(NSUP)):
                        o_ps = p2o.tile([97, 512], F32, tag="ops", name="ops")
                        nkt = 4 * Q + 4
                        for kt in range(nkt):
                            s_ps = p2ps.tile([128, 512], F32, tag="sps",
                                             name="sps")
                            nc.tensor.matmul(
                                s_ps[:], kT[:, 128 * kt:128 * (kt + 1)],
                                qT[:, 512 * Q:512 * (Q + 1)],
                                start=True, stop=True)
                            e_sb = p2t.tile([128, 512], F32R, tag="esb",
                                            name="esb")
                            nc.scalar.activation(out=e_sb[:], in_=s_ps[:],
                                                 func=act.Exp)
                            dj = kt - 4 * Q
                            if dj >= 0:
                                nc.vector.tensor_tensor(
                                    out=e_sb[:], in0=e_sb[:].bitcast(F32),
                                    in1=masks_sb[:, 512 * dj:512 * (dj + 1)],
                                    op=alu.mult)
                            nc.tensor.matmul(o_ps[:], v_sb[:, kt, :], e_sb[:],
                                             start=(kt == 0),
                                             stop=(kt == nkt - 1))
                        j = 4 * bb_ + Q
                        o_u = p2w.tile([96, 512], F32, tag=f"ou{j}",
                                       name=f"ou{j}")
                        nc.vector.tensor_copy(out=o_u[:], in_=o_ps[0:96, :])
                        nc.vector.tensor_copy(
                            out=den_all[96:97, 512 * j:512 * (j + 1)],
                            in_=o_ps[96:97, :])
                        o_all[j] = o_u

                # batched normalization: one partition-scatter DMA, one
                # reciprocal over 8 lanes, then per-unit bcast + multiply
                d8 = p2w.tile([8, 512], F32, tag="d8")
                for j in range(8):
                    nc.sync.dma_start(
                        out=d8[j:j + 1, :],
                        in_=den_all[96:97, 512 * j:512 * (j + 1)])
                with nc.allow_low_precision(reason="fp32r bcast rhs"):
                    nc.vector.reciprocal(out=rec8[:], in_=d8[:])
                recrow = p2w.tile([1, 8 * 512], F32R, tag="recrow")
                for j in range(8):
                    nc.sync.dma_start(
                        out=recrow[0:1, 512 * j:512 * (j + 1)],
                        in_=rec8[j:j + 1, :])
                for j in range(8):
                    rb = p2rb.tile([96, 512], F32, tag="rb")
                    nc.tensor.matmul(rb[:], ones_sb[:, 0:96],
                                     recrow[0:1, 512 * j:512 * (j + 1)],
                                     start=True, stop=True)
                    onrm = p2t.tile([96, 512], F32, tag="onrm")
                    nc.vector.tensor_tensor(out=onrm[:], in0=o_all[j][:],
                                            in1=rb[:], op=alu.mult)
                    nc.scalar.dma_start(
                        out=slab2_in[96 * j:96 * (j + 1), :], in_=onrm[:])

            nc.gpsimd.collective_compute(
                "AllToAll", mybir.AluOpType.bypass,
                replica_groups=[list(range(N_CORES))],
                ins=[slab2_in[:].opt()], outs=[slab2_out[:].opt()])

            # ================= PHASE 3 =================
            with contextlib.ExitStack() as p3:
                p3w = p3.enter_context(tc.tile_pool(name="p3w", bufs=1))
                p3t = p3.enter_context(tc.tile_pool(name="p3t", bufs=2))
                p3s = p3.enter_context(tc.tile_pool(name="p3s", bufs=2))
                p3ps = p3.enter_context(
                    tc.tile_pool(name="p3ps", bufs=2, space="PSUM"))

                x2 = []
                x2r = []
                for k in range(KD):
                    o_sb = p3t.tile([128, TPC], F32, tag="osb")
                    nc.sync.dma_start(out=o_sb[:],
                                      in_=slab2_out[128 * k:128 * (k + 1), :])
                    t = p3w.tile([128, TPC], F32, tag=f"x2_{k}")
                    nc.vector.tensor_tensor(out=t[:], in0=o_sb[:],
                                            in1=xT[k][:].bitcast(F32),
                                            op=alu.add)
                    x2.append(t)
                    tr = p3w.tile([128, TPC], F32R, tag=f"x2r{k}")
                    nc.vector.tensor_copy(out=tr[:], in_=t[:])
                    x2r.append(tr)

                a2, be2 = layernorm_stats(p3t, x2r, "l2")
                x2n = ln_apply(p3w, p3t, [t[:] for t in x2], a2, be2,
                               g2_sb, b2_sb, "l2a")

                b1a_sb = p3w.tile([128, 16], F32, tag="b1a")
                nc.sync.dma_start(out=b1a_sb[:], in_=b1a_d[:])
                b1g_sb = p3w.tile([128, 16], F32, tag="b1g")
                nc.sync.dma_start(out=b1g_sb[:], in_=b1g_d[:])
                bf2_sb = p3w.tile([128, KD], F32, tag="bf2")
                nc.sync.dma_start(out=bf2_sb[:], in_=bf2_d[:])

                # fc1: weight tiles fetched as (128, 512) four-slice groups
                a_tiles = []
                sw = []
                with tc.tile_pool(name="p3h", bufs=2, space="PSUM") as p3h:
                    for g in range(8):           # g<4: a-half, g>=4: gate-half
                        w1g = []
                        for k in range(KD):
                            t = p3s.tile([128, 512], F32R, tag=f"w1g{k}")
                            nc.sync.dma_start(
                                out=t[:],
                                in_=w1_d[128 * k:128 * (k + 1),
                                         512 * g:512 * (g + 1)])
                            w1g.append(t)
                        for mi in range(4):
                            i = 4 * (g % 4) + mi
                            h_ps = p3h.tile([128, TPC], F32, tag="hps")
                            for k in range(KD):
                                nc.tensor.matmul(
                                    h_ps[:],
                                    w1g[k][:, 128 * mi:128 * (mi + 1)],
                                    x2n[k][:],
                                    start=(k == 0), stop=(k == KD - 1))
                            if g < 4:
                                a_sb = p3w.tile([128, TPC], F32, tag=f"a{i}")
                                nc.vector.tensor_scalar(
                                    out=a_sb[:], in0=h_ps[:],
                                    scalar1=b1a_sb[:, i:i + 1],
                                    scalar2=None, op0=alu.add)
                                a_tiles.append(a_sb)
                            else:
                                sil = p3t.tile([128, TPC], F32, tag="sil")
                                nc.scalar.activation(
                                    out=sil[:], in_=h_ps[:], func=act.Silu,
                                    bias=b1g_sb[:, i:i + 1])
                                swt = p3w.tile([128, TPC], F32R, tag=f"sw{i}")
                                nc.vector.tensor_tensor(
                                    out=swt[:], in0=sil[:],
                                    in1=a_tiles[i][:], op=alu.mult)
                                sw.append(swt)

                # fc2: k2-outer, 6 persistent ff psum banks
                with tc.tile_pool(name="p3f", bufs=1, space="PSUM") as p3f:
                    ff_ps = [p3f.tile([128, TPC], F32, tag=f"ff{d}",
                                      name=f"ff{d}")
                             for d in range(KD)]
                    for k2 in range(16):
                        w2c = p3s.tile([128, D], F32R, tag="w2c")
                        nc.sync.dma_start(
                            out=w2c[:],
                            in_=w2_d[128 * k2:128 * (k2 + 1), :])
                        for d in range(KD):
                            nc.tensor.matmul(ff_ps[d][:],
                                             w2c[:, 128 * d:128 * (d + 1)],
                                             sw[k2][:],
                                             start=(k2 == 0), stop=(k2 == 15))
                    for d in range(KD):
                        t = p3t.tile([128, TPC], F32, tag="fft")
                        nc.vector.tensor_scalar(out=t[:], in0=ff_ps[d][:],
                                                scalar1=bf2_sb[:, d:d + 1],
                                                scalar2=None, op0=alu.add)
                        o = p3t.tile([128, TPC], F32, tag="oout")
                        nc.vector.tensor_tensor(out=o[:], in0=t[:],
                                                in1=x2[d][:], op=alu.add)
                        nc.sync.dma_start(
                            out=outT_d[128 * d:128 * (d + 1), :], in_=o[:])

    nc.compile()
    _prog_cache[key] = nc
    return nc


def _host_inputs(x_type, x_value, seq_order, W_attn, type_emb, g1, b1, g2, b2,
                 W_fc1, b_fc1, W_fc2, b_fc2):
    f32 = np.float32
    x_type = np.asarray(x_type)
    seq_order = np.asarray(seq_order)
    x_value = np.asarray(x_value, dtype=f32)
    W_attn = np.asarray(W_attn, dtype=f32)
    type_emb = np.asarray(type_emb, dtype=f32)
    W_fc1 = np.asarray(W_fc1, dtype=f32)
    W_fc2 = np.asarray(W_fc2, dtype=f32)
    g1 = np.asarray(g1, f32); b1 = np.asarray(b1, f32)
    g2 = np.asarray(g2, f32); b2 = np.asarray(b2, f32)
    b_fc1 = np.asarray(b_fc1, f32); b_fc2 = np.asarray(b_fc2, f32)

    wqk_full = W_attn[:, :1536][:, QK_PERM].copy()
    te_full = type_emb[:, QK_PERM]  # (16, 1536) values for each qk feature
    q_origin = QK_PERM < 768
    te_q = np.where(q_origin[None, :], te_full, 0.0).astype(f32)
    te_k = np.where(~q_origin[None, :], te_full, 0.0).astype(f32)

    invf16 = (1.0 / THETA ** (np.arange(0, DR, 2, dtype=f32) / DR)).astype(f32)
    invf_col = invf16[(np.arange(128) % 32) // 2].reshape(128, 1)

    # masks: block (128k x 512q), mask[kk, qq] = 1 if qq >= kk + 128*dj
    kk = np.arange(128)[:, None]
    qq = np.arange(512)[None, :]
    masks = np.concatenate(
        [(qq >= kk + 128 * dj).astype(f32) for dj in range(4)], axis=1)

    # rot lhsT: lhsT[k, m] = P[m, k];  P[2i, 2i+1] = -1, P[2i+1, 2i] = +1
    R = np.zeros((128, 128), f32)
    for i in range(64):
        R[2 * i + 1, 2 * i] = -1.0
        R[2 * i, 2 * i + 1] = 1.0
    B4m = np.zeros((4, 128), f32)
    B4m[0, 0:32] = 1.0; B4m[1, 32:64] = 1.0
    B4m[2, 64:96] = 1.0; B4m[3, 96:128] = 1.0

    common = {
        "Wqk": wqk_full, "Wv": W_attn[:, 1536:].copy(),
        "te_q": te_q, "te_k": te_k,
        "invf": invf_col,
        "g1c": g1.reshape(6, 128).T.copy(), "b1c": b1.reshape(6, 128).T.copy(),
        "g2c": g2.reshape(6, 128).T.copy(), "b2c": b2.reshape(6, 128).T.copy(),
        "W1": W_fc1, "W2": W_fc2,
        "b1a": b_fc1[:2048].reshape(16, 128).T.copy(),
        "b1g": b_fc1[2048:].reshape(16, 128).T.copy(),
        "bf2": b_fc2.reshape(6, 128).T.copy(),
        "masks": masks, "R128": R, "B4": B4m,
        "ones128": np.ones((1, 128), f32),
        "onescol": np.ones((128, 1), f32),
        "iota16": np.arange(16, dtype=f32).reshape(16, 1),
        "epsc": np.full((1, 1), EPS, f32),
    }
    in_maps = []
    for c in range(N_CORES):
        b = c // 4
        t0 = 512 * (c % 4)
        m = dict(common)
        m["xT"] = np.ascontiguousarray(x_value[b, t0:t0 + TPC, :].T)
        m["qtype"] = x_type[b, t0:t0 + TPC].astype(f32).reshape(1, TPC)
        m["ktype"] = x_type[b, t0 + 1:t0 + TPC + 1].astype(f32).reshape(1, TPC)
        pos4 = np.stack([
            seq_order[0, b, t0:t0 + TPC],
            seq_order[1, b, t0:t0 + TPC],
            seq_order[0, b, t0 + 1:t0 + TPC + 1],
            seq_order[1, b, t0 + 1:t0 + TPC + 1],
        ]).astype(f32)
        m["pos4"] = pos4
        in_maps.append(m)
    return in_maps


def kernel(**inputs):
    nc = build_program()
    in_maps = _host_inputs(**inputs)
    res = run_bass_kernel_spmd(nc, in_maps, list(range(N_CORES)), trace=False)
    out = np.empty((B, T, D), np.float32)
    for c in range(N_CORES):
        b = c // 4
        t0 = 512 * (c % 4)
        out[b, t0:t0 + TPC, :] = res.results[c]["outT"].T
    return out



# revision 6
# speedup vs baseline: 1.2525x; 1.2525x over previous
"""Trainium2 Bass kernel for nn_DecoderLayer_90967407329666.

Decoder layer: LN1 -> QKV (+type emb) -> multi-axis RoPE -> causal SDPA
-> residual -> LN2 -> SwiGLU FFN -> residual.  B=2, T=2048, D=768, H=8,
DFF=2048, NTYPE=16, NAX=2 rotary axes of 32 dims each.

Sharding (8 cores):
  Phase 1 (token-parallel): each core owns 512 tokens (core c: batch c//4,
    tokens 512*(c%4)...) and computes LN1 + q,k (feature-major) + v
    (token-major) + type-emb + RoPE for those tokens, all 8 heads.
  ONE merged AllToAll: 288-row block per head (q 0:64 rope | q tail 64:96 |
    k rope 96:160 | k tail 160:192 | v token-major 192:288), bf16.
  Phase 2 (head-parallel): core c runs full causal attention for head c,
    both batches; exp trimmed to the causal column range on diagonal
    blocks; softmax sums via an appended ones-column on v.
  AllToAll #2: o goes back token-parallel (bf16).
  Phase 3 (token-parallel): residual + LN2 + SwiGLU FFN + residual.

All weights and matmul operands are bf16 (full PE rate); LN gains/biases
are folded into the weights host-side; the residual stream, LN stats and
softmax denominators stay fp32.  W_fc1/W_fc2 are prefetched at kernel
start so their HBM traffic hides under phase 1 + the first AllToAll.
"""

import sys

sys.path.insert(0, "/opt/trn_rl_repo")

import numpy as np

import contextlib

import concourse.bacc as bacc
import concourse.bass as bass
import concourse.tile as tile
from concourse import mybir
from concourse.bass_utils import run_bass_kernel_spmd

# ---- problem constants (hardcoded per contest rules) ----
B, T = 2, 2048
D, H, DFF, NTYPE = 768, 8, 2048, 16
NAX = 2
HD = D // H            # 96
DR = HD // (NAX + 1)   # 32
EPS = 1e-5
THETA = 10000.0
N_CORES = 8
TPC = 512              # tokens per core
NSUP = 4               # supertiles per batch (2048/512)
KD = D // 128          # 6 contraction chunks over D
SCALE = 1.0 / np.sqrt(np.float32(HD))
BLK = 288              # slab rows per head block (q96 + k96 + v96)

F32 = mybir.dt.float32
F32R = mybir.dt.float32r
BF = mybir.dt.bfloat16
I32 = mybir.dt.int32
NPBF = mybir.dt.np(BF)

# Cody-Waite split of 2*pi (C1 has 12 mantissa bits -> n*C1 exact for n<2^11)
C1 = float(np.float32(np.floor(2 * np.pi * 2**9) / 2**9))
C2 = float(np.float32(2 * np.pi - C1))
C3 = float(np.float32(2 * np.pi - C1 - float(np.float32(2 * np.pi - C1))))
HALF_PI = float(np.pi / 2)

# qk output-feature permutation: 12 slices of 128 rows
#   slices 0..7  : [q_h dims 0:64 | k_h dims 0:64]   (rope rows)
#   slice  8, 9  : q tails (dims 64:96) of heads 0..3 / 4..7
#   slice 10,11  : k tails of heads 0..3 / 4..7
def _qk_colperm():
    cols = []
    for h in range(H):
        cols += list(range(96 * h, 96 * h + 64))          # q_h 0:64
        cols += list(range(768 + 96 * h, 768 + 96 * h + 64))  # k_h 0:64
    for h in range(H):
        cols += list(range(96 * h + 64, 96 * h + 96))     # q tails
    for h in range(H):
        cols += list(range(768 + 96 * h + 64, 768 + 96 * h + 96))  # k tails
    return np.array(cols)

QK_PERM = _qk_colperm()

_prog_cache = {}


def build_program():
    key = ("v2",)
    if key in _prog_cache:
        return _prog_cache[key]
    nc = bacc.Bacc("TRN2", target_bir_lowering=False, debug=False,
                   num_devices=N_CORES)
    alu = mybir.AluOpType
    act = mybir.ActivationFunctionType

    # ---------------- DRAM I/O ----------------
    xT_d = nc.dram_tensor("xT", [D, TPC], F32R, kind="ExternalInput")
    wqk_d = nc.dram_tensor("Wqk", [D, 1536], BF, kind="ExternalInput")
    wv_d = nc.dram_tensor("Wv", [D, D], BF, kind="ExternalInput")
    teq_d = nc.dram_tensor("te_q", [NTYPE, 1536], BF, kind="ExternalInput")
    tek_d = nc.dram_tensor("te_k", [NTYPE, 1536], BF, kind="ExternalInput")
    bwv_d = nc.dram_tensor("bWv", [1, D], BF, kind="ExternalInput")
    qtype_d = nc.dram_tensor("qtype", [1, TPC], F32R, kind="ExternalInput")
    ktype_d = nc.dram_tensor("ktype", [1, TPC], F32R, kind="ExternalInput")
    pos4_d = nc.dram_tensor("pos4", [4, TPC], F32R, kind="ExternalInput")
    invf_d = nc.dram_tensor("invf", [128, 1], F32, kind="ExternalInput")
    w1_d = nc.dram_tensor("W1", [D, 2 * DFF], BF, kind="ExternalInput")
    w2_d = nc.dram_tensor("W2", [DFF, D], BF, kind="ExternalInput")
    b1a_d = nc.dram_tensor("b1a", [128, 16], F32, kind="ExternalInput")
    b1g_d = nc.dram_tensor("b1g", [128, 16], F32, kind="ExternalInput")
    bf2_d = nc.dram_tensor("bf2", [128, KD], F32, kind="ExternalInput")
    tri_d = nc.dram_tensor("tri", [128, 128], BF, kind="ExternalInput")
    r128_d = nc.dram_tensor("R128", [128, 128], BF, kind="ExternalInput")
    b4_d = nc.dram_tensor("B4", [4, 128], F32R, kind="ExternalInput")
    ones_d = nc.dram_tensor("ones128", [1, 128], F32R, kind="ExternalInput")
    onesbf_d = nc.dram_tensor("onesbf", [1, 128], BF, kind="ExternalInput")
    onescol_d = nc.dram_tensor("onescol", [128, 1], F32R, kind="ExternalInput")
    onescolbf_d = nc.dram_tensor("onescolbf", [128, 1], BF,
                                 kind="ExternalInput")
    iota_d = nc.dram_tensor("iota16", [16, 1], F32, kind="ExternalInput")
    eps_d = nc.dram_tensor("epsc", [1, 1], F32, kind="ExternalInput")
    outT_d = nc.dram_tensor("outT", [D, TPC], F32, kind="ExternalOutput")

    with tile.TileContext(nc) as tc:
        with tc.tile_pool(name="glob", bufs=1) as glob, \
             tc.tile_pool(name="dram", bufs=1, space="DRAM") as dram:
            # exchange slabs (bf16)
            slab_in = dram.tile([8 * BLK, TPC], BF, tag="slab_in")
            slab_out = dram.tile([8 * BLK, TPC], BF, tag="slab_out")
            slab2_in = dram.tile([D, TPC], BF, tag="slab2_in")
            slab2_out = dram.tile([D, TPC], BF, tag="slab2_out")

            # ---- persistent constants / activations ----
            ones_sb = glob.tile([1, 128], F32R, tag="ones")
            nc.sync.dma_start(out=ones_sb[:], in_=ones_d[:])
            ones_bf = glob.tile([1, 128], BF, tag="onesbf")
            nc.sync.dma_start(out=ones_bf[:], in_=onesbf_d[:])
            onescol_sb = glob.tile([128, 1], F32R, tag="onescol")
            nc.sync.dma_start(out=onescol_sb[:], in_=onescol_d[:])
            onescol_bf = glob.tile([128, 1], BF, tag="onescolbf")
            nc.sync.dma_start(out=onescol_bf[:], in_=onescolbf_d[:])
            iota_sb = glob.tile([16, 1], F32, tag="iota")
            nc.sync.dma_start(out=iota_sb[:], in_=iota_d[:])
            eps_sb = glob.tile([1, 1], F32, tag="eps")
            nc.sync.dma_start(out=eps_sb[:], in_=eps_d[:])
            xT = []
            for k in range(KD):
                t = glob.tile([128, TPC], F32R, tag=f"xT{k}")
                nc.sync.dma_start(out=t[:], in_=xT_d[128 * k:128 * (k + 1), :])
                xT.append(t)
            tri_sb = glob.tile([128, 128], BF, tag="tri")
            nc.sync.dma_start(out=tri_sb[:], in_=tri_d[:])
            b1a_sb = glob.tile([128, 16], F32, tag="b1a")
            nc.sync.dma_start(out=b1a_sb[:], in_=b1a_d[:])
            b1g_sb = glob.tile([128, 16], F32, tag="b1g")
            nc.sync.dma_start(out=b1g_sb[:], in_=b1g_d[:])
            bf2_sb = glob.tile([128, KD], F32, tag="bf2")
            nc.sync.dma_start(out=bf2_sb[:], in_=bf2_d[:])

            def layernorm_stats(pool, persist, src_tiles, tag):
                """src (fp32r views, 6 x (128,TPC)) -> (alpha, beta) rows."""
                ctx = contextlib.ExitStack()
                ps_pool = ctx.enter_context(
                    tc.tile_pool(name=f"{tag}ps", bufs=1, space="PSUM"))
                sums = ps_pool.tile([1, TPC], F32, tag=f"{tag}sums")
                sumsq = ps_pool.tile([1, TPC], F32, tag=f"{tag}sumsq")
                for k in range(KD):
                    sq = pool.tile([128, TPC], F32R, tag="lnsq", bufs=2)
                    nc.vector.tensor_tensor(
                        out=sq[:], in0=src_tiles[k].bitcast(F32),
                        in1=src_tiles[k].bitcast(F32), op=alu.mult)
                    nc.tensor.matmul(sums[:], onescol_sb[:], src_tiles[k],
                                     start=(k == 0), stop=(k == KD - 1))
                    nc.tensor.matmul(sumsq[:], onescol_sb[:], sq[:],
                                     start=(k == 0), stop=(k == KD - 1))
                mean = pool.tile([1, TPC], F32, tag="lnmean")
                nc.vector.tensor_scalar(out=mean[:], in0=sums[:],
                                        scalar1=1.0 / D, scalar2=None,
                                        op0=alu.mult)
                var = pool.tile([1, TPC], F32, tag="lnvar")
                nc.vector.tensor_tensor(out=var[:], in0=mean[:], in1=mean[:],
                                        op=alu.mult)
                nc.vector.scalar_tensor_tensor(
                    out=var[:], in0=sumsq[:], scalar=1.0 / D, in1=var[:],
                    op0=alu.mult, op1=alu.subtract)
                std = pool.tile([1, TPC], F32, tag="lnstd")
                nc.scalar.activation(out=std[:], in_=var[:], func=act.Sqrt,
                                     bias=eps_sb[:])
                alpha = persist.tile([1, TPC], F32R, tag=f"{tag}alpha")
                with nc.allow_low_precision(reason="fp32r bcast rhs"):
                    nc.vector.reciprocal(out=alpha[:], in_=std[:])
                beta = persist.tile([1, TPC], F32R, tag=f"{tag}beta")
                nc.vector.scalar_tensor_tensor(
                    out=beta[:], in0=mean[:], scalar=-1.0,
                    in1=alpha[:].bitcast(F32), op0=alu.mult, op1=alu.mult)
                ctx.close()
                return alpha, beta

            def ln_apply(pool, scratch, src_f32_aps, alpha, beta, tag):
                """out[k] = src*ab + bb  (g/b folded into weights), bf16."""
                ctx = contextlib.ExitStack()
                ps_pool = ctx.enter_context(
                    tc.tile_pool(name=f"{tag}ps", bufs=1, space="PSUM"))
                ab = ps_pool.tile([128, TPC], F32, tag=f"{tag}ab")
                nc.tensor.matmul(ab[:], ones_sb[:], alpha[:], start=True,
                                 stop=True)
                bb = ps_pool.tile([128, TPC], F32, tag=f"{tag}bb")
                nc.tensor.matmul(bb[:], ones_sb[:], beta[:], start=True,
                                 stop=True)
                outs = []
                for k in range(KD):
                    t1 = scratch.tile([128, TPC], F32, tag="lnt1", bufs=2)
                    nc.vector.tensor_tensor(out=t1[:], in0=src_f32_aps[k],
                                            in1=ab[:], op=alu.mult)
                    o = pool.tile([128, TPC], BF, tag=f"{tag}o{k}")
                    nc.vector.tensor_tensor(out=o[:], in0=t1[:], in1=bb[:],
                                            op=alu.add)
                    outs.append(o)
                ctx.close()
                return outs

            # ================= PHASE 1 =================
            with contextlib.ExitStack() as p1:
                p1t = p1.enter_context(tc.tile_pool(name="p1t", bufs=1))
                p1w = p1.enter_context(tc.tile_pool(name="p1w", bufs=1))
                p1qk = p1.enter_context(
                    tc.tile_pool(name="p1qk", bufs=1, space="PSUM"))

                # phase-1 weights (freed at the AllToAll)
                wqk = []
                for k in range(KD):
                    t = p1w.tile([128, 1536], BF, tag=f"wqk{k}")
                    nc.sync.dma_start(out=t[:],
                                      in_=wqk_d[128 * k:128 * (k + 1), :])
                    wqk.append(t)
                wv = []
                for k in range(KD):
                    t = p1w.tile([128, D], BF, tag=f"wv{k}")
                    nc.sync.dma_start(out=t[:],
                                      in_=wv_d[128 * k:128 * (k + 1), :])
                    wv.append(t)
                teq_sb = p1w.tile([NTYPE, 1536], BF, tag="teq")
                nc.sync.dma_start(out=teq_sb[:], in_=teq_d[:])
                tek_sb = p1w.tile([NTYPE, 1536], BF, tag="tek")
                nc.sync.dma_start(out=tek_sb[:], in_=tek_d[:])
                bwv_sb = p1w.tile([1, D], BF, tag="bwv")
                nc.sync.dma_start(out=bwv_sb[:], in_=bwv_d[:])
                r128_sb = p1w.tile([128, 128], BF, tag="r128")
                nc.sync.dma_start(out=r128_sb[:], in_=r128_d[:])
                b4_sb = p1w.tile([4, 128], F32R, tag="b4")
                nc.sync.dma_start(out=b4_sb[:], in_=b4_d[:])
                invf_sb = p1w.tile([128, 1], F32, tag="invf")
                nc.sync.dma_start(out=invf_sb[:], in_=invf_d[:])
                pos4_sb = p1w.tile([4, TPC], F32R, tag="pos4")
                nc.sync.dma_start(out=pos4_sb[:], in_=pos4_d[:])
                qt_sb = p1w.tile([1, TPC], F32R, tag="qt")
                nc.sync.dma_start(out=qt_sb[:], in_=qtype_d[:])
                kt_sb = p1w.tile([1, TPC], F32R, tag="kt")
                nc.sync.dma_start(out=kt_sb[:], in_=ktype_d[:])
                # FFN weights prefetched now (after phase-1 loads in the
                # sync queue): traffic hides under phase 1 + AllToAll
                w1t = []
                for k in range(KD):
                    t = glob.tile([128, 2 * DFF], BF, tag=f"w1_{k}")
                    nc.sync.dma_start(out=t[:],
                                      in_=w1_d[128 * k:128 * (k + 1), :])
                    w1t.append(t)
                w2t = []
                for k2 in range(16):
                    t = glob.tile([128, D], BF, tag=f"w2_{k2}")
                    nc.sync.dma_start(out=t[:],
                                      in_=w2_d[128 * k2:128 * (k2 + 1), :])
                    w2t.append(t)

                # LN1
                a1, be1 = layernorm_stats(p1t, p1w, [x[:] for x in xT], "l1")
                xn = ln_apply(p1w, p1t, [x[:].bitcast(F32) for x in xT],
                              a1, be1, "l1a")

                # one-hot type codes (16, TPC)
                p1misc = p1.enter_context(
                    tc.tile_pool(name="p1misc", bufs=1, space="PSUM"))

                def onehot(row_sb, tag):
                    bc = p1misc.tile([16, TPC], F32, tag="ohbc")
                    nc.tensor.matmul(bc[:], ones_sb[:, 0:16], row_sb[:],
                                     start=True, stop=True)
                    oh = p1w.tile([16, TPC], BF, tag=f"{tag}oh")
                    nc.vector.tensor_scalar(out=oh[:], in0=bc[:],
                                            scalar1=iota_sb[:], scalar2=None,
                                            op0=alu.is_equal)
                    return oh
                oh_q = onehot(qt_sb, "q")
                oh_k = onehot(kt_sb, "k")

                # cos/sin tiles (128, TPC): rows 0:64 q-axes, 64:128 k-axes
                pm = p1misc.tile([128, TPC], F32, tag="pm")
                nc.tensor.matmul(pm[:], b4_sb[:], pos4_sb[:], start=True,
                                 stop=True)
                f_t = p1t.tile([128, TPC], F32, tag="f")
                nc.vector.tensor_scalar(out=f_t[:], in0=pm[:],
                                        scalar1=invf_sb[:], scalar2=None,
                                        op0=alu.mult)
                nt = p1t.tile([128, TPC], F32, tag="nt")
                nc.vector.tensor_scalar(out=nt[:], in0=f_t[:],
                                        scalar1=float(1.0 / (2 * np.pi)),
                                        scalar2=None, op0=alu.mult)
                n_i = p1t.tile([128, TPC], I32, tag="ni")
                nc.vector.tensor_copy(out=n_i[:], in_=nt[:])
                nc.vector.tensor_copy(out=nt[:], in_=n_i[:])
                # fr computed in place on f_t (Cody-Waite range reduction)
                nc.vector.scalar_tensor_tensor(out=f_t[:], in0=nt[:],
                                               scalar=-C1, in1=f_t[:],
                                               op0=alu.mult, op1=alu.add)
                nc.vector.scalar_tensor_tensor(out=f_t[:], in0=nt[:],
                                               scalar=-C2, in1=f_t[:],
                                               op0=alu.mult, op1=alu.add)
                nc.vector.scalar_tensor_tensor(out=f_t[:], in0=nt[:],
                                               scalar=-C3, in1=f_t[:],
                                               op0=alu.mult, op1=alu.add)
                s_t = p1t.tile([128, TPC], F32, tag="sin")
                nc.scalar.activation(out=s_t[:], in_=f_t[:], func=act.Sin)
                nc.scalar.activation(out=nt[:], in_=f_t[:], func=act.Abs)
                nc.vector.tensor_scalar(out=nt[:], in0=nt[:], scalar1=-1.0,
                                        scalar2=HALF_PI, op0=alu.mult,
                                        op1=alu.add)
                c_t = p1t.tile([128, TPC], F32, tag="cos")
                nc.scalar.activation(out=c_t[:], in_=nt[:], func=act.Sin)
                # fold score scale 1/sqrt(HD) into q: scale c,s rows 0:64
                nc.vector.tensor_scalar(out=c_t[0:64, :], in0=c_t[0:64, :],
                                        scalar1=float(SCALE), scalar2=None,
                                        op0=alu.mult)
                nc.vector.tensor_scalar(out=s_t[0:64, :], in0=s_t[0:64, :],
                                        scalar1=float(SCALE), scalar2=None,
                                        op0=alu.mult)
                c_b = p1w.tile([128, TPC], BF, tag="cosb")
                nc.vector.tensor_copy(out=c_b[:], in_=c_t[:])
                s_b = p1w.tile([128, TPC], BF, tag="sinb")
                nc.vector.tensor_copy(out=s_b[:], in_=s_t[:])

                # qk slices: matmul + type emb, then rope / tails -> slab
                for s in range(12):
                    qk_ps = p1qk.tile([128, TPC], F32, tag="qkps", bufs=2)
                    for k in range(KD):
                        nc.tensor.matmul(qk_ps[:],
                                         wqk[k][:, 128 * s:128 * (s + 1)],
                                         xn[k][:], start=(k == 0), stop=False)
                    nc.tensor.matmul(qk_ps[:],
                                     teq_sb[:, 128 * s:128 * (s + 1)],
                                     oh_q[:], start=False, stop=False)
                    nc.tensor.matmul(qk_ps[:],
                                     tek_sb[:, 128 * s:128 * (s + 1)],
                                     oh_k[:], start=False, stop=True)
                    if s < 8:
                        # rope: q_h 0:64 | k_h 0:64
                        rsb = p1t.tile([128, TPC], BF, tag="rsb", bufs=2)
                        nc.vector.tensor_copy(out=rsb[:], in_=qk_ps[:])
                        rot = p1qk.tile([128, TPC], F32, tag="rot", bufs=2)
                        nc.tensor.matmul(rot[:], r128_sb[:], rsb[:],
                                         start=True, stop=True)
                        t1 = p1t.tile([128, TPC], BF, tag="rt1", bufs=2)
                        nc.vector.tensor_tensor(out=t1[:], in0=rsb[:],
                                                in1=c_b[:], op=alu.mult)
                        t2 = p1t.tile([128, TPC], BF, tag="rt2", bufs=2)
                        nc.vector.tensor_tensor(out=t2[:], in0=rot[:],
                                                in1=s_b[:], op=alu.mult)
                        qkr = p1t.tile([128, TPC], BF, tag="qkr", bufs=2)
                        nc.vector.tensor_tensor(out=qkr[:], in0=t1[:],
                                                in1=t2[:], op=alu.add)
                        h = s
                        nc.scalar.dma_start(
                            out=slab_in[BLK * h + 0:BLK * h + 64, :],
                            in_=qkr[0:64, :])
                        nc.scalar.dma_start(
                            out=slab_in[BLK * h + 96:BLK * h + 160, :],
                            in_=qkr[64:128, :])
                    else:
                        # tails: s=8,9 q tails h0..3/h4..7 (scaled); 10,11 k
                        tl = p1t.tile([128, TPC], BF, tag="tail", bufs=2)
                        sc = float(SCALE) if s < 10 else 1.0
                        nc.vector.tensor_scalar(out=tl[:], in0=qk_ps[:],
                                                scalar1=sc, scalar2=None,
                                                op0=alu.mult)
                        base = 64 if s < 10 else 160
                        h0 = 4 * (s % 2)
                        dst = bass.AP(
                            tensor=slab_in[:].tensor,
                            offset=(BLK * h0 + base) * TPC,
                            ap=[[BLK * TPC, 4], [TPC, 32], [1, TPC]])
                        nc.scalar.dma_start(out=dst, in_=tl[:])

                # v (token-major): 4 tok-slices x 2 halves of 384 cols
                for ts_ in range(4):
                    for hf in range(2):
                        v_ps = p1qk.tile([128, 384], F32, tag="vps", bufs=2)
                        for k in range(KD):
                            nc.tensor.matmul(
                                v_ps[:],
                                xn[k][:, 128 * ts_:128 * (ts_ + 1)],
                                wv[k][:, 384 * hf:384 * (hf + 1)],
                                start=(k == 0), stop=False)
                        nc.tensor.matmul(
                            v_ps[:], ones_bf[:],
                            bwv_sb[:, 384 * hf:384 * (hf + 1)],
                            start=False, stop=True)
                        v_sb1 = p1t.tile([128, 384], BF, tag="vsb1", bufs=2)
                        nc.vector.tensor_copy(out=v_sb1[:], in_=v_ps[:])
                        # one DMA for 4 heads: [tok 128][head 4][feat 96]
                        dst = bass.AP(
                            tensor=slab_in[:].tensor,
                            offset=(BLK * 4 * hf + 192) * TPC
                            + 128 * ts_ * 96,
                            ap=[[96, 128], [BLK * TPC, 4], [1, 96]])
                        nc.scalar.dma_start(out=dst, in_=v_sb1[:])

                nc.gpsimd.collective_compute(
                    "AllToAll", mybir.AluOpType.bypass,
                    replica_groups=[list(range(N_CORES))],
                    ins=[slab_in[:].opt()],
                    outs=[slab_out[:].opt()])

            # ================= PHASE 2 =================
            with contextlib.ExitStack() as p2:
                p2w = p2.enter_context(tc.tile_pool(name="p2w", bufs=1))
                p2t = p2.enter_context(tc.tile_pool(name="p2t", bufs=3))
                p2ps = p2.enter_context(
                    tc.tile_pool(name="p2ps", bufs=4, space="PSUM"))
                p2o = p2.enter_context(
                    tc.tile_pool(name="p2o", bufs=2, space="PSUM"))
                p2rb = p2.enter_context(
                    tc.tile_pool(name="p2rb", bufs=2, space="PSUM"))

                den_all = p2w.tile([1, 8 * 512], F32, tag="denall")
                rec8 = p2w.tile([8, 512], F32R, tag="rec8")
                o_all = {}

                for bb_ in range(2):
                    qT = p2w.tile([96, 2048], BF, tag=f"qT{bb_}")
                    kT = p2w.tile([96, 2048], BF, tag=f"kT{bb_}")
                    v_sb = p2w.tile([128, 16, 97], BF, tag=f"v{bb_}")
                    ones_bc = bass.AP(
                        tensor=onescol_bf[:].tensor,
                        offset=onescol_bf[:].offset,
                        ap=[[1, 128], [0, 16], [0, 1]])
                    nc.sync.dma_start(out=v_sb[:, :, 96:97], in_=ones_bc)
                    # batched unpack: q / k one DMA each, v one per supertile
                    src_q = bass.AP(
                        tensor=slab_out[:].tensor,
                        offset=BLK * 4 * bb_ * TPC,
                        ap=[[TPC, 96], [BLK * TPC, 4], [1, TPC]])
                    nc.sync.dma_start(out=qT[:], in_=src_q)
                    src_k = bass.AP(
                        tensor=slab_out[:].tensor,
                        offset=(BLK * 4 * bb_ + 96) * TPC,
                        ap=[[TPC, 96], [BLK * TPC, 4], [1, TPC]])
                    nc.sync.dma_start(out=kT[:], in_=src_k)
                    for u in range(4):
                        src_v = bass.AP(
                            tensor=slab_out[:].tensor,
                            offset=(BLK * (4 * bb_ + u) + 192) * TPC,
                            ap=[[96, 128], [128 * 96, 4], [1, 96]])
                        nc.sync.dma_start(out=v_sb[:, 4 * u:4 * u + 4, 0:96],
                                          in_=src_v)

                    for Q in reversed(range(NSUP)):
                        o_ps = p2o.tile([97, 512], F32, tag="ops", name="ops")
                        nkt = 4 * Q + 4
                        for kt in range(nkt):
                            s_ps = p2ps.tile([128, 512], F32, tag="sps",
                                             name="sps")
                            nc.tensor.matmul(
                                s_ps[:], kT[:, 128 * kt:128 * (kt + 1)],
                                qT[:, 512 * Q:512 * (Q + 1)],
                                start=True, stop=True)
                            e_sb = p2t.tile([128, 512], BF, tag="esb",
                                            name="esb")
                            dj = kt - 4 * Q
                            if dj > 0:
                                # cols < 128*dj fully masked: zero, skip exp
                                nc.vector.memset(e_sb[:, 0:128 * dj], 0.0)
                            lo = 128 * dj if dj > 0 else 0
                            nc.scalar.activation(out=e_sb[:, lo:],
                                                 in_=s_ps[:, lo:],
                                                 func=act.Exp)
                            if dj >= 0:
                                # triangular strip: one 128-col mask mult
                                nc.vector.tensor_tensor(
                                    out=e_sb[:, lo:lo + 128],
                                    in0=e_sb[:, lo:lo + 128],
                                    in1=tri_sb[:], op=alu.mult)
                            nc.tensor.matmul(o_ps[:], v_sb[:, kt, :], e_sb[:],
                                             start=(kt == 0),
                                             stop=(kt == nkt - 1))
                        j = 4 * bb_ + Q
                        o_u = p2w.tile([96, 512], BF, tag=f"ou{j}",
                                       name=f"ou{j}")
                        nc.vector.tensor_copy(out=o_u[:], in_=o_ps[0:96, :])
                        nc.vector.tensor_copy(
                            out=den_all[0:1, 512 * j:512 * (j + 1)],
                            in_=o_ps[96:97, :])
                        o_all[j] = o_u

                # batched normalization
                d8 = p2w.tile([8, 512], F32, tag="d8")
                for j in range(8):
                    nc.sync.dma_start(
                        out=d8[j:j + 1, :],
                        in_=den_all[0:1, 512 * j:512 * (j + 1)])
                with nc.allow_low_precision(reason="fp32r bcast rhs"):
                    nc.vector.reciprocal(out=rec8[:], in_=d8[:])
                recrow = p2w.tile([1, 8 * 512], F32R, tag="recrow")
                for j in range(8):
                    nc.sync.dma_start(
                        out=recrow[0:1, 512 * j:512 * (j + 1)],
                        in_=rec8[j:j + 1, :])
                for j in range(8):
                    rb = p2rb.tile([96, 512], F32, tag="rb")
                    nc.tensor.matmul(rb[:], ones_sb[:, 0:96],
                                     recrow[0:1, 512 * j:512 * (j + 1)],
                                     start=True, stop=True)
                    onrm = p2t.tile([96, 512], BF, tag="onrm")
                    nc.vector.tensor_tensor(out=onrm[:], in0=o_all[j][:],
                                            in1=rb[:], op=alu.mult)
                    nc.scalar.dma_start(
                        out=slab2_in[96 * j:96 * (j + 1), :], in_=onrm[:])

            nc.gpsimd.collective_compute(
                "AllToAll", mybir.AluOpType.bypass,
                replica_groups=[list(range(N_CORES))],
                ins=[slab2_in[:].opt()], outs=[slab2_out[:].opt()])

            # ================= PHASE 3 =================
            with contextlib.ExitStack() as p3:
                p3w = p3.enter_context(tc.tile_pool(name="p3w", bufs=1))
                p3t = p3.enter_context(tc.tile_pool(name="p3t", bufs=2))
                x2 = []
                for k in range(KD):
                    o_sb = p3t.tile([128, TPC], BF, tag="osb")
                    nc.sync.dma_start(out=o_sb[:],
                                      in_=slab2_out[128 * k:128 * (k + 1), :])
                    t = p3w.tile([128, TPC], F32R, tag=f"x2_{k}")
                    nc.vector.tensor_tensor(out=t[:], in0=o_sb[:],
                                            in1=xT[k][:].bitcast(F32),
                                            op=alu.add)
                    x2.append(t)

                a2, be2 = layernorm_stats(
                    p3t, p3w, [t[:] for t in x2], "l2")
                x2n = ln_apply(p3w, p3t,
                               [t[:].bitcast(F32) for t in x2], a2, be2,
                               "l2a")

                # fc1 (weights preloaded in w1t)
                a_tiles = []
                sw = []
                with tc.tile_pool(name="p3h", bufs=2, space="PSUM") as p3h:
                    for g in range(8):           # g<4: a-half, g>=4: gate
                        for mi in range(4):
                            i = 4 * (g % 4) + mi
                            col = 512 * g + 128 * mi
                            h_ps = p3h.tile([128, TPC], F32, tag="hps")
                            for k in range(KD):
                                nc.tensor.matmul(
                                    h_ps[:],
                                    w1t[k][:, col:col + 128],
                                    x2n[k][:],
                                    start=(k == 0), stop=(k == KD - 1))
                            if g < 4:
                                a_sb = p3w.tile([128, TPC], BF, tag=f"a{i}")
                                nc.vector.tensor_scalar(
                                    out=a_sb[:], in0=h_ps[:],
                                    scalar1=b1a_sb[:, i:i + 1],
                                    scalar2=None, op0=alu.add)
                                a_tiles.append(a_sb)
                            else:
                                sil = p3t.tile([128, TPC], BF, tag="sil")
                                nc.scalar.activation(
                                    out=sil[:], in_=h_ps[:], func=act.Silu,
                                    bias=b1g_sb[:, i:i + 1])
                                swt = p3w.tile([128, TPC], BF, tag=f"sw{i}")
                                nc.vector.tensor_tensor(
                                    out=swt[:], in0=sil[:],
                                    in1=a_tiles[i][:], op=alu.mult)
                                sw.append(swt)

                # fc2: k2-outer, 6 persistent ff psum banks (weights in w2t)
                with tc.tile_pool(name="p3f", bufs=1, space="PSUM") as p3f:
                    ff_ps = [p3f.tile([128, TPC], F32, tag=f"ff{d}",
                                      name=f"ff{d}")
                             for d in range(KD)]
                    for k2 in range(16):
                        for d in range(KD):
                            nc.tensor.matmul(ff_ps[d][:],
                                             w2t[k2][:, 128 * d:128 * (d + 1)],
                                             sw[k2][:],
                                             start=(k2 == 0), stop=(k2 == 15))
                    for d in range(KD):
                        t = p3t.tile([128, TPC], F32, tag="fft")
                        nc.vector.tensor_scalar(out=t[:], in0=ff_ps[d][:],
                                                scalar1=bf2_sb[:, d:d + 1],
                                                scalar2=None, op0=alu.add)
                        o = p3t.tile([128, TPC], F32, tag="oout")
                        nc.vector.tensor_tensor(out=o[:], in0=t[:],
                                                in1=x2[d][:].bitcast(F32),
                                                op=alu.add)
                        nc.sync.dma_start(
                            out=outT_d[128 * d:128 * (d + 1), :], in_=o[:])

    nc.compile()
    _prog_cache[key] = nc
    return nc


def _host_inputs(x_type, x_value, seq_order, W_attn, type_emb, g1, b1, g2, b2,
                 W_fc1, b_fc1, W_fc2, b_fc2):
    f32 = np.float32
    x_type = np.asarray(x_type)
    seq_order = np.asarray(seq_order)
    x_value = np.asarray(x_value, dtype=f32)
    W_attn = np.asarray(W_attn, dtype=f32)
    type_emb = np.asarray(type_emb, dtype=f32)
    W_fc1 = np.asarray(W_fc1, dtype=f32)
    W_fc2 = np.asarray(W_fc2, dtype=f32)
    g1 = np.asarray(g1, f32); b1 = np.asarray(b1, f32)
    g2 = np.asarray(g2, f32); b2 = np.asarray(b2, f32)
    b_fc1 = np.asarray(b_fc1, f32); b_fc2 = np.asarray(b_fc2, f32)

    # fold LN gains/biases into the weights:
    #   qkv = LN(x)@W = (xhat*g1 + b1)@W = xhat@(g1[:,None]*W) + b1@W
    Wg = W_attn * g1[:, None]
    bW = b1 @ W_attn                       # (2304,)
    wqk_full = Wg[:, :1536][:, QK_PERM].copy()
    te_full = type_emb[:, QK_PERM]         # (16, 1536)
    bW_qk = bW[:1536][QK_PERM]
    q_origin = QK_PERM < 768
    te_q = np.where(q_origin[None, :], te_full + bW_qk[None, :], 0.0)
    te_k = np.where(~q_origin[None, :], te_full + bW_qk[None, :], 0.0)
    bWv = bW[1536:].reshape(1, D)

    W1g = W_fc1 * g2[:, None]
    b_fc1_eff = b_fc1 + b2 @ W_fc1         # (4096,)

    invf16 = (1.0 / THETA ** (np.arange(0, DR, 2, dtype=f32) / DR)).astype(f32)
    invf_col = invf16[(np.arange(128) % 32) // 2].reshape(128, 1)

    # triangular 128x128 strip mask: tri[kk, c] = 1 if c >= kk
    kk = np.arange(128)[:, None]
    cc = np.arange(128)[None, :]
    tri = (cc >= kk).astype(NPBF)

    # rot lhsT: lhsT[k, m] = P[m, k];  P[2i, 2i+1] = -1, P[2i+1, 2i] = +1
    R = np.zeros((128, 128), f32)
    for i in range(64):
        R[2 * i + 1, 2 * i] = -1.0
        R[2 * i, 2 * i + 1] = 1.0
    B4m = np.zeros((4, 128), f32)
    B4m[0, 0:32] = 1.0; B4m[1, 32:64] = 1.0
    B4m[2, 64:96] = 1.0; B4m[3, 96:128] = 1.0

    common = {
        "Wqk": wqk_full.astype(NPBF), "Wv": Wg[:, 1536:].astype(NPBF),
        "te_q": te_q.astype(NPBF), "te_k": te_k.astype(NPBF),
        "bWv": bWv.astype(NPBF),
        "invf": invf_col,
        "W1": W1g.astype(NPBF), "W2": W_fc2.astype(NPBF),
        "b1a": b_fc1_eff[:2048].reshape(16, 128).T.copy(),
        "b1g": b_fc1_eff[2048:].reshape(16, 128).T.copy(),
        "bf2": b_fc2.reshape(6, 128).T.copy(),
        "tri": tri, "R128": R.astype(NPBF), "B4": B4m,
        "ones128": np.ones((1, 128), f32),
        "onesbf": np.ones((1, 128), NPBF),
        "onescol": np.ones((128, 1), f32),
        "onescolbf": np.ones((128, 1), NPBF),
        "iota16": np.arange(16, dtype=f32).reshape(16, 1),
        "epsc": np.full((1, 1), EPS, f32),
    }
    in_maps = []
    for c in range(N_CORES):
        b = c // 4
        t0 = 512 * (c % 4)
        m = dict(common)
        m["xT"] = np.ascontiguousarray(x_value[b, t0:t0 + TPC, :].T)
        m["qtype"] = x_type[b, t0:t0 + TPC].astype(f32).reshape(1, TPC)
        m["ktype"] = x_type[b, t0 + 1:t0 + TPC + 1].astype(f32).reshape(1, TPC)
        pos4 = np.stack([
            seq_order[0, b, t0:t0 + TPC],
            seq_order[1, b, t0:t0 + TPC],
            seq_order[0, b, t0 + 1:t0 + TPC + 1],
            seq_order[1, b, t0 + 1:t0 + TPC + 1],
        ]).astype(f32)
        m["pos4"] = pos4
        in_maps.append(m)
    return in_maps


def kernel(**inputs):
    nc = build_program()
    in_maps = _host_inputs(**inputs)
    res = run_bass_kernel_spmd(nc, in_maps, list(range(N_CORES)), trace=False)
    out = np.empty((B, T, D), np.float32)
    for c in range(N_CORES):
        b = c // 4
        t0 = 512 * (c % 4)
        out[b, t0:t0 + TPC, :] = res.results[c]["outT"].T
    return out
